# revision 3
# baseline (speedup 1.0000x reference)
"""NT-Xent loss on 8 Trainium2 NeuronCores.

Strategy (per core c):
  - Host rotates z = concat(z_i, z_j) by -1024*c rows, so every core runs the
    IDENTICAL program on "local rows 0..1023": diag col of local row i is i,
    positive col is i + 4096. One compiled NEFF, SPMD across 8 cores.
  - Phase 1 is a streamed pipeline over 8 row-groups (1024 rows each):
    cast-load f32->bf16 (SWDGE), per-row sumsq (DVE stt fused mult+reduce),
    inv_norm = exp(-0.5*ln(sumsq)) (ACT), zn = z * inv_norm (DVE), store to a
    DRAM scratch, then xbar DMA-transpose of that group's rows into the
    column slice of znT [256, 8192] bf16 (two [128, 8192] SBUF tiles).
  - Main loop is q-major so matmuls start once the first two groups are
    transposed: bf16 matmuls znT_m.T @ znT -> sim tiles in PSUM [128, 2048]
    windows; ACT computes exp(2*sim) with fused free-axis accumulation (row
    sums); DVE extracts diag/pos sim values from PSUM pre-exp via an
    identity-mask fused reduce.
  - Tail: denom = rowsum - exp(2*diag); term = ln(denom) - 2*pos; partition
    reduction via ones-matmul -> per-core scalar partial.
  - Host sums the 8 partials / 8192.
"""

import os
import sys

sys.path.insert(0, "/opt/trn_rl_repo")
os.environ.setdefault("MYCRO_LOCAL_CACHE", "1")

import numpy as np

import concourse.bass as bass
import concourse.mybir as mybir
from concourse import bacc, tile
from concourse.bass_utils import run_bass_kernel_spmd

F32 = mybir.dt.float32
BF16 = mybir.dt.bfloat16
AF = mybir.ActivationFunctionType
ALU = mybir.AluOpType

N_CORES = 8
TWO_N = 8192
D = 256
P = 128
NCHUNK = TWO_N // P               # 64 row-chunks of 128
NGROUPS = 8                       # phase-1 pipeline groups
GCHUNK = NCHUNK // NGROUPS        # 8 chunks per group
GROWS = TWO_N // NGROUPS          # 1024 rows per group
ROWS_PER_CORE = TWO_N // N_CORES  # 1024
M_CHUNKS = ROWS_PER_CORE // P     # 8 local row chunks
NCOL = 512                        # matmul free dim (one PSUM bank)
QCOL = 2048                       # ACT window = 4 banks
N_Q = TWO_N // QCOL               # 4
POS_OFF = TWO_N // 2              # 4096
TEMP_SCALE = 2.0                  # 1 / temperature

_NC_CACHE = {}


def _build_nc():
    nc = bacc.Bacc(
        "TRN2",
        target_bir_lowering=False,
        debug=False,
        enable_asserts=False,
        num_devices=N_CORES,
    )
    z = nc.dram_tensor("z", [TWO_N, D], F32, kind="ExternalInput")
    ident = nc.dram_tensor("ident", [P, P], F32, kind="ExternalInput")
    ones = nc.dram_tensor("ones", [P, 1], F32, kind="ExternalInput")
    out = nc.dram_tensor("partial", [1, 1], F32, kind="ExternalOutput")
    zs = nc.dram_tensor("zscratch", [TWO_N, D], BF16, kind="Internal")

    with tile.TileContext(nc) as tc:
        with (
            tc.tile_pool(name="big", bufs=1) as big,
            tc.tile_pool(name="gpool", bufs=3) as gpool,
            tc.tile_pool(name="spool", bufs=3) as spool,
            tc.tile_pool(name="work", bufs=2) as work,
            tc.tile_pool(name="small", bufs=1) as small,
        ):
            id_sb = big.tile([P, P], F32)
            nc.sync.dma_start(id_sb[:], ident[:])
            ones_sb = big.tile([P, 1], F32)
            nc.sync.dma_start(ones_sb[:], ones[:])

            zt0 = big.tile([P, TWO_N], BF16)
            zt1 = big.tile([P, TWO_N], BF16)
            zts = [zt0, zt1]

            zv = z[:].rearrange("(n p) d -> p n d", p=P)
            zsv = zs[:].rearrange("(n p) d -> p n d", p=P)

            # ---- phase 1: streamed load + normalize + transpose ----
            for g in range(NGROUPS):
                sl = slice(g * GCHUNK, (g + 1) * GCHUNK)
                zbg = gpool.tile([P, GCHUNK, D], BF16, tag="zb")
                nc.gpsimd.dma_start(zbg[:], zv[:, sl, :])
                ssg = spool.tile([P, GCHUNK], F32, tag="ss")
                for c in range(GCHUNK):
                    scr = work.tile([P, D], BF16, tag="sqscr")
                    nc.vector.scalar_tensor_tensor(
                        out=scr[:], in0=zbg[:, c, :], scalar=1.0,
                        in1=zbg[:, c, :], op0=ALU.mult, op1=ALU.mult,
                        accum_out=ssg[:, c:c + 1])
                lng = spool.tile([P, GCHUNK], F32, tag="lng")
                nc.scalar.activation(lng[:], ssg[:], AF.Ln)
                invg = spool.tile([P, GCHUNK], F32, tag="invg")
                nc.scalar.activation(invg[:], lng[:], AF.Exp, scale=-0.5)
                znbg = gpool.tile([P, GCHUNK, D], BF16, tag="znb")
                for c in range(GCHUNK):
                    nc.vector.tensor_scalar_mul(
                        znbg[:, c, :], zbg[:, c, :], invg[:, c:c + 1])
                nc.sync.dma_start(zsv[:, sl, :], znbg[:])
                rsl = slice(g * GROWS, (g + 1) * GROWS)
                csl = slice(g * GROWS, (g + 1) * GROWS)
                nc.sync.dma_start_transpose(zt0[:, csl], zs[rsl, 0:P])
                nc.sync.dma_start_transpose(zt1[:, csl], zs[rsl, P:2 * P])

            # ---- phase 2: sim matmul + exp row-sums (q-major) ----
            sums = small.tile([P, M_CHUNKS * N_Q], F32)
            diag = small.tile([P, M_CHUNKS], F32)
            pos = small.tile([P, M_CHUNKS], F32)

            with tc.tile_pool(name="psum", bufs=2, space="PSUM") as psum_pool:
                for q in range(N_Q):
                    for m in range(M_CHUNKS):
                        pt = psum_pool.tile([P, QCOL], F32, tag="sim")
                        for k in range(2):
                            lhsT = zts[k][:, m * P:(m + 1) * P]
                            for nn in range(QCOL // NCOL):
                                col = q * QCOL + nn * NCOL
                                nc.tensor.matmul(
                                    pt[:, nn * NCOL:(nn + 1) * NCOL],
                                    lhsT,
                                    zts[k][:, col:col + NCOL],
                                    start=(k == 0),
                                    stop=(k == 1),
                                )
                        # extract diag / positive similarity pre-exp
                        if q == 0:
                            scr = work.tile([P, P], F32, tag="extr")
                            nc.vector.scalar_tensor_tensor(
                                out=scr[:], in0=pt[:, m * P:m * P + P],
                                scalar=1.0, in1=id_sb[:],
                                op0=ALU.mult, op1=ALU.mult,
                                accum_out=diag[:, m:m + 1])
                        if q == POS_OFF // QCOL:  # q == 2: positive block
                            scr = work.tile([P, P], F32, tag="extr")
                            nc.vector.scalar_tensor_tensor(
                                out=scr[:], in0=pt[:, m * P:m * P + P],
                                scalar=1.0, in1=id_sb[:],
                                op0=ALU.mult, op1=ALU.mult,
                                accum_out=pos[:, m:m + 1])
                        # exp(2*sim) with fused row-sum accumulation
                        et = work.tile([P, QCOL], BF16, tag="expbuf")
                        col_ix = m * N_Q + q
                        nc.scalar.activation(
                            et[:], pt[:], AF.Exp, scale=TEMP_SCALE,
                            accum_out=sums[:, col_ix:col_ix + 1])

            # ---- tail: per-core partial loss ----
            dexp = small.tile([P, M_CHUNKS], F32)
            nc.scalar.activation(dexp[:], diag[:], AF.Exp, scale=TEMP_SCALE)
            stot = small.tile([P, M_CHUNKS], F32)
            nc.vector.tensor_reduce(
                stot[:],
                sums[:].rearrange("p (m q) -> p m q", q=N_Q),
                axis=mybir.AxisListType.X,
                op=ALU.add,
            )
            denom = small.tile([P, M_CHUNKS], F32)
            nc.vector.tensor_sub(denom[:], stot[:], dexp[:])
            lnd = small.tile([P, M_CHUNKS], F32)
            nc.scalar.activation(lnd[:], denom[:], AF.Ln)
            term = small.tile([P, M_CHUNKS], F32)
            tsum = small.tile([P, 1], F32)
            nc.vector.scalar_tensor_tensor(
                out=term[:], in0=pos[:], scalar=-TEMP_SCALE, in1=lnd[:],
                op0=ALU.mult, op1=ALU.add, accum_out=tsum[:])
            with tc.tile_pool(name="psum2", bufs=1, space="PSUM") as pp2:
                pfin = pp2.tile([1, 1], F32)
                nc.tensor.matmul(pfin[:], ones_sb[:], tsum[:],
                                 start=True, stop=True)
                res = small.tile([1, 1], F32)
                nc.vector.tensor_copy(res[:], pfin[:])
                nc.sync.dma_start(out[:], res[:])

    nc.compile()
    return nc


def _get_nc():
    if "nc" not in _NC_CACHE:
        _NC_CACHE["nc"] = _build_nc()
    return _NC_CACHE["nc"]


def _prepare_in_maps(z_i, z_j):
    z_full = np.concatenate(
        [np.asarray(z_i, np.float32), np.asarray(z_j, np.float32)], axis=0
    )
    ident = np.eye(P, dtype=np.float32)
    ones = np.ones((P, 1), dtype=np.float32)
    in_maps = []
    for c in range(N_CORES):
        zc = np.roll(z_full, -ROWS_PER_CORE * c, axis=0)
        in_maps.append({"z": np.ascontiguousarray(zc), "ident": ident, "ones": ones})
    return in_maps


def kernel(z_i, z_j):
    nc = _get_nc()
    in_maps = _prepare_in_maps(z_i, z_j)
    res = run_bass_kernel_spmd(nc, in_maps, core_ids=list(range(N_CORES)))
    total = 0.0
    for c in range(N_CORES):
        total += float(res.results[c]["partial"][0, 0])
    loss = total / float(TWO_N)
    return np.float32(loss)


if __name__ == "__main__":
    rng = np.random.default_rng(0)
    z_i = rng.standard_normal((4096, 256), dtype=np.float32)
    z_j = rng.standard_normal((4096, 256), dtype=np.float32)
    print("loss:", kernel(z_i, z_j))
